# revision 19
# baseline (speedup 1.0000x reference)
"""Trainium2 Bass kernel for KernelSelfAttn (linear attention) distributed over 8 cores.

Math (per reference):
  h1 = x@W1 + b1 ; q,k = h1[:, :1024], h1[:, 1024:2048]; non_att = h1[:, 2048:]
  v = x@Wv + bv
  per head (8 heads, dh=dv=128):
    qf = elu(q)+1 = exp(min(q,0)) + relu(q)   (same for k)
    kv = kf^T @ v ; k_sum = kf.sum(n)         -> reductions over N (all-reduced)
    att = (qf @ kv) / (qf @ k_sum)
  out = non_att + att_cat @ Wo + bo           (biases are all zero per spec)

Sharding: rows of x split across 8 cores; [kv | k_sum] ([128, 1032] fp32)
all-reduced; everything else local.

Precision strategy: the output is non_att + att@Wo where the att contribution
is ~1e-3 of the non_att scale, so the whole attention path tolerates fp8.
 - non_att gemm (x@W1na): bf16 lhsT/rhs (precision-critical).
 - k, v, q gemms and att@Wo: fp8e4 operands with DoubleRow perf mode
   (2 contraction rows/cycle). fp8 weights are host-prescaled by 32 (W ~
   N(0,1/1024) would be subnormal in e4m3); the 1/32 descale folds into the
   activation-engine ops that already run. att@Wo additionally scales the
   normalizer by 1024 so att(~7e-4) lands in e4m3's normal range; the
   combined 1/(32*1024) descale folds into the PSUM-evacuation copy.
 - kv/k_sum/att/qk matmuls stay bf16 (small, and keep the feature pipeline
   simple). All accumulation is fp32 in PSUM; AllReduce in fp32.

Layouts on chip:
  xT   [din-part, n]  bf16 + fp8 copies, SBUF-resident (96KB/part)
  k,v  [n-part, dim]  natural (contraction for kv needs n on partitions)
  qT   [dqk-part, n]  transposed (contraction for att needs dh on partitions)
  attT [dv-part, n]   feeds output projection as stationary operand
"""

import sys

import numpy as np

sys.path.insert(0, "/opt/trn_rl_repo")

DIN = 1024
DQK = 1024
DV = 1024
H = 8
DH = 128
NCORES = 8
N_FULL = 32768
NS = N_FULL // NCORES  # 4096 rows per core
BLK = 512
NBLK = NS // BLK  # 8
CPB = BLK // 128  # chunks (of 128 rows) per block

WS = 32.0  # host pre-scale on fp8 weights
AS = 1024.0  # normalizer pre-scale so att fits e4m3
_cache = {}


def _build_bass(no_collective=False, reps=1):
    """reps>1 unrolls the whole kernel body back-to-back inside one NEFF.
    Used by the benchmark to measure the marginal (steady-state) HW cost of
    one kernel execution, net of the fixed per-launch dispatch overhead."""
    import concourse.bass as bass
    import concourse.mybir as mybir
    import concourse.tile as tile
    from concourse import bacc
    from concourse.masks import make_identity
    from contextlib import ExitStack

    fp32 = mybir.dt.float32
    bf16 = mybir.dt.bfloat16
    fp8 = mybir.dt.float8e4
    AF = mybir.ActivationFunctionType
    ALU = mybir.AluOpType
    DR = mybir.MatmulPerfMode.DoubleRow

    nc = bacc.Bacc(None)

    x = nc.declare_dram_parameter("x", [NS, DIN], bf16, isOutput=False)
    w1k8 = nc.declare_dram_parameter("w1k8", [DIN, DQK], fp8, isOutput=False)
    w1q8 = nc.declare_dram_parameter("w1q8", [DIN, DQK], fp8, isOutput=False)
    wv8 = nc.declare_dram_parameter("wv8", [DIN, DV], fp8, isOutput=False)
    wo8 = nc.declare_dram_parameter("wo8", [DV, DIN], fp8, isOutput=False)
    w1na = nc.declare_dram_parameter("w1na", [DIN, DIN], bf16, isOutput=False)
    out = nc.declare_dram_parameter("out", [NS, DIN], fp32, isOutput=True)

    KVW = H * DH + H  # 1032: [kv (8*128) | k_sum (8)]

    with ExitStack() as top:
        tc = top.enter_context(tile.TileContext(nc))

        consts = top.enter_context(tc.tile_pool(name="consts", bufs=1))
        ident = consts.tile([128, 128], bf16)
        make_identity(nc, ident[:])
        ones = consts.tile([128, 1], bf16)
        nc.gpsimd.memset(ones[:], 1.0)

        # row-selector weights: sel[:, h*128:(h+1)*128] is [8,128] with row h
        # all-ones -> K=8 matmul broadcasts rall[h, :] across 128 partitions
        sel = consts.tile([8, H * 128], bf16)
        sel_i = consts.tile([8, H * 128], mybir.dt.int32)
        nc.gpsimd.iota(
            sel_i[:].rearrange("p (h w) -> p h w", w=128),
            pattern=[[1, H], [0, 128]],
            base=0,
            channel_multiplier=-1,
        )
        nc.vector.tensor_scalar(sel[:], sel_i[:], 0, None, ALU.is_equal)

        dram = top.enter_context(tc.tile_pool(name="dram", bufs=1, space="DRAM"))
        kv_in = dram.tile([128, KVW], fp32)
        kv_out = dram.tile([128, KVW], fp32)

        def emit_rep(rep):
            with ExitStack() as body:
                # xT tiles: one per block, written in phase 1, read by both
                # phases. bf16 copy feeds non_att; fp8 copy feeds k/v/q gemms.
                xt_pool = body.enter_context(tc.tile_pool(name="xt", bufs=1))
                xT = [
                    xt_pool.tile([128, 8, BLK], bf16, name=f"xT{b}", tag=f"xT{b}")
                    for b in range(NBLK)
                ]
                xT8 = [
                    xt_pool.tile([128, 8, BLK], fp8, name=f"xT8_{b}", tag=f"xT8_{b}")
                    for b in range(NBLK)
                ]

                # all weight loads issued up front so phase 2 never waits on DMA
                w_pool = body.enter_context(tc.tile_pool(name="wts", bufs=1))
                w1k = w_pool.tile([128, 8, 1024], fp8, name="w1k", tag="w1k")
                wv = w_pool.tile([128, 8, 1024], fp8, name="wv", tag="wv")
                w1q = w_pool.tile([128, 8, 1024], fp8, name="w1q", tag="w1q")
                w1n = w_pool.tile([128, 8, 1024], bf16, name="w1n", tag="w1n")
                wo = w_pool.tile([128, 8, 1024], fp8, name="wo", tag="wo")
                for d in range(8):
                    nc.sync.dma_start(w1k[:, d, :], w1k8[d * 128 : (d + 1) * 128, :])
                    nc.sync.dma_start(wv[:, d, :], wv8[d * 128 : (d + 1) * 128, :])
                    nc.sync.dma_start(w1q[:, d, :], w1q8[d * 128 : (d + 1) * 128, :])
                    nc.sync.dma_start(w1n[:, d, :], w1na[d * 128 : (d + 1) * 128, :])
                    nc.sync.dma_start(wo[:, d, :], wo8[d * 128 : (d + 1) * 128, :])

                # ---------------- Phase 1: xT, k, v, kv ----------------
                with ExitStack() as p1:

                    # kv / k_sum accumulators live in PSUM across all of phase 1
                    psum_kv = p1.enter_context(
                        tc.tile_pool(name="psum_kv", bufs=1, space="PSUM")
                    )
                    kv_acc = [
                        psum_kv.tile([128, 512], fp32, name="kva0", tag="kva0"),
                        psum_kv.tile([128, 512], fp32, name="kva1", tag="kva1"),
                    ]
                    ks_acc = psum_kv.tile([128, 8], fp32, name="ksa", tag="ksa")

                    xin_pool = p1.enter_context(tc.tile_pool(name="xin", bufs=8))
                    kf_pool = p1.enter_context(tc.tile_pool(name="kfeat", bufs=2))
                    va_pool = p1.enter_context(tc.tile_pool(name="vaug", bufs=2))
                    t1_pool = p1.enter_context(tc.tile_pool(name="p1tmp", bufs=4))
                    psum_t = p1.enter_context(
                        tc.tile_pool(name="psum_t", bufs=2, space="PSUM")
                    )
                    psum_g = p1.enter_context(
                        tc.tile_pool(name="psum_g", bufs=3, space="PSUM")
                    )

                    for b in range(NBLK):
                        xin = []
                        for c in range(CPB):
                            t = xin_pool.tile([128, DIN], bf16)
                            r0 = b * BLK + c * 128
                            nc.sync.dma_start(t[:], x[r0 : r0 + 128, :])
                            xin.append(t)

                        for d in range(8):
                            tp = psum_t.tile([128, BLK], bf16)
                            for c in range(CPB):
                                nc.tensor.transpose(
                                    tp[:, c * 128 : (c + 1) * 128],
                                    xin[c][:, d * 128 : (d + 1) * 128],
                                    ident[:],
                                )
                            nc.vector.tensor_copy(xT[b][:, d, :], tp[:])
                            nc.gpsimd.tensor_copy(xT8[b][:, d, :], xT[b][:, d, :])

                        for c in range(CPB):
                            kf = kf_pool.tile([128, 1024], bf16)
                            va = va_pool.tile([128, 1024], bf16)
                            for s in range(2):  # k halves (psum holds 32*k)
                                ps = psum_g.tile([128, 512], fp32)
                                for d in range(0, 8, 2):
                                    nc.tensor.matmul(
                                        ps[:],
                                        xT8[b][:, d : d + 2, c * 128 : (c + 1) * 128],
                                        w1k[:, d : d + 2, s * 512 : (s + 1) * 512],
                                        start=(d == 0),
                                        stop=(d == 6),
                                        perf_mode=DR,
                                    )
                                # feature map: exp(min(k,0)) + max(k,0), k = ps/32
                                tmp = t1_pool.tile([128, 512], bf16, tag="texp")
                                nc.scalar.activation(
                                    tmp[:], ps[:], AF.Relu, scale=-1.0 / WS
                                )
                                nc.scalar.activation(tmp[:], tmp[:], AF.Exp, scale=-1.0)
                                mx = t1_pool.tile([128, 512], bf16, tag="tmax")
                                nc.scalar.activation(
                                    mx[:], ps[:], AF.Relu, scale=1.0 / WS
                                )
                                nc.vector.tensor_add(
                                    kf[:, s * 512 : (s + 1) * 512], mx[:], tmp[:]
                                )
                            for s in range(2):  # v halves (psum holds 32*v)
                                ps = psum_g.tile([128, 512], fp32)
                                for d in range(0, 8, 2):
                                    nc.tensor.matmul(
                                        ps[:],
                                        xT8[b][:, d : d + 2, c * 128 : (c + 1) * 128],
                                        wv[:, d : d + 2, s * 512 : (s + 1) * 512],
                                        start=(d == 0),
                                        stop=(d == 6),
                                        perf_mode=DR,
                                    )
                                nc.vector.tensor_scalar_mul(
                                    va[:, s * 512 : (s + 1) * 512], ps[:], 1.0 / WS
                                )
                            first = b == 0 and c == 0
                            last = b == NBLK - 1 and c == CPB - 1
                            for h in range(H):
                                nc.tensor.matmul(
                                    kv_acc[h // 4][
                                        :, (h % 4) * 128 : (h % 4 + 1) * 128
                                    ],
                                    kf[:, h * 128 : (h + 1) * 128],
                                    va[:, h * 128 : (h + 1) * 128],
                                    start=first,
                                    stop=last,
                                )
                                nc.tensor.matmul(
                                    ks_acc[:, h : h + 1],
                                    kf[:, h * 128 : (h + 1) * 128],
                                    ones[:],
                                    start=first,
                                    stop=last,
                                )

                    # evacuate [kv | k_sum] to DRAM bounce buffer
                    kv_sb = kf_pool.tile([128, KVW], fp32, name="kv_sb", tag="kv_sb")
                    nc.vector.tensor_copy(kv_sb[:, 0:512], kv_acc[0][:])
                    nc.vector.tensor_copy(kv_sb[:, 512:1024], kv_acc[1][:])
                    nc.vector.tensor_copy(kv_sb[:, 1024:1032], ks_acc[:])
                    nc.sync.dma_start(kv_in[:], kv_sb[:])

                # ---------------- AllReduce [kv | k_sum] ----------------
                if no_collective:  # timeline-sim variant: local copy, no AllReduce
                    nc.sync.dma_start(kv_out[:], kv_in[:])
                else:
                    nc.gpsimd.collective_compute(
                        "AllReduce",
                        mybir.AluOpType.add,
                        replica_groups=[list(range(NCORES))],
                        ins=[kv_in.opt()],
                        outs=[kv_out.opt()],
                    )
                kvp = body.enter_context(tc.tile_pool(name="kvpost", bufs=1))
                kv2 = kvp.tile([128, KVW], fp32, name="kv2", tag="kv2")
                nc.sync.dma_start(kv2[:], kv_out[:])
                kv_bf = kvp.tile([128, 1024], bf16, name="kv_bf", tag="kv_bf")
                nc.vector.tensor_copy(kv_bf[:], kv2[:, 0:1024])

                # block-diagonal k_sum for the qk matmul: ks[:, h*8+h] = k_sum_h
                ks_sb = kvp.tile([128, 64], bf16, name="ks_sb", tag="ks_sb")
                nc.gpsimd.memset(ks_sb[:], 0.0)
                for h in range(H):
                    nc.vector.tensor_copy(
                        ks_sb[:, h * 8 + h : h * 8 + h + 1],
                        kv2[:, 1024 + h : 1024 + h + 1],
                    )

                # ---------------- Phase 2: q, att, out ----------------
                with ExitStack() as p2:
                    w2_pool = p2.enter_context(tc.tile_pool(name="w2", bufs=1))
                    w1q = w2_pool.tile([128, 8, 1024], fp8, name="w1q", tag="w1q")
                    w1n = w2_pool.tile([128, 8, 1024], bf16, name="w1n", tag="w1n")
                    wo = w2_pool.tile([128, 8, 1024], fp8, name="wo", tag="wo")
                    for d in range(8):
                        nc.sync.dma_start(w1q[:, d, :], w1q8[d * 128 : (d + 1) * 128, :])
                        nc.sync.dma_start(w1n[:, d, :], w1na[d * 128 : (d + 1) * 128, :])
                        nc.sync.dma_start(wo[:, d, :], wo8[d * 128 : (d + 1) * 128, :])

                    qf_pool = p2.enter_context(tc.tile_pool(name="qf", bufs=2))
                    an_pool = p2.enter_context(tc.tile_pool(name="an", bufs=2))
                    t2_pool = p2.enter_context(tc.tile_pool(name="p2tmp", bufs=4))
                    rr_pool = p2.enter_context(tc.tile_pool(name="rall", bufs=2))
                    out_pool = p2.enter_context(tc.tile_pool(name="osb", bufs=3))
                    psum_q = p2.enter_context(
                        tc.tile_pool(name="psum_q", bufs=2, space="PSUM")
                    )
                    psum_k = p2.enter_context(
                        tc.tile_pool(name="psum_k", bufs=1, space="PSUM")
                    )
                    psum_a = p2.enter_context(
                        tc.tile_pool(name="psum_a", bufs=2, space="PSUM")
                    )
                    psum_b = p2.enter_context(
                        tc.tile_pool(name="psum_b", bufs=1, space="PSUM")
                    )
                    psum_o = p2.enter_context(
                        tc.tile_pool(name="psum_o", bufs=1, space="PSUM")
                    )

                    for b in range(NBLK):
                        qf = qf_pool.tile([128, H, BLK], bf16)  # [p(dh), head, n]
                        for qh in range(H):
                            qp = psum_q.tile([128, BLK], fp32)
                            for d in range(0, 8, 2):
                                nc.tensor.matmul(
                                    qp[:],
                                    w1q[:, d : d + 2, qh * 128 : (qh + 1) * 128],
                                    xT8[b][:, d : d + 2, :],
                                    start=(d == 0),
                                    stop=(d == 6),
                                    perf_mode=DR,
                                )
                            tmp = t2_pool.tile([128, BLK], bf16, tag="texp")
                            nc.scalar.activation(tmp[:], qp[:], AF.Relu, scale=-1.0 / WS)
                            nc.scalar.activation(tmp[:], tmp[:], AF.Exp, scale=-1.0)
                            mx = t2_pool.tile([128, BLK], bf16, tag="tmax")
                            nc.scalar.activation(mx[:], qp[:], AF.Relu, scale=1.0 / WS)
                            nc.vector.tensor_add(qf[:, qh, :], mx[:], tmp[:])

                        qkp = psum_k.tile([8, BLK], fp32)
                        for h in range(H):
                            nc.tensor.matmul(
                                qkp[:],
                                ks_sb[:, h * 8 : (h + 1) * 8],
                                qf[:, h, :],
                                start=(h == 0),
                                stop=(h == H - 1),
                            )
                        rall = rr_pool.tile([8, BLK], fp32, tag="rall_f")
                        nc.vector.reciprocal(rall[:], qkp[:])
                        rbf = rr_pool.tile([8, BLK], bf16, tag="rall_b")
                        nc.vector.tensor_copy(rbf[:], rall[:])

                        # attT, normalized and pre-scaled by AS, cast to fp8
                        an = an_pool.tile([128, H, BLK], fp8)
                        for h in range(H):
                            ap_ = psum_a.tile([128, BLK], fp32)
                            nc.tensor.matmul(
                                ap_[:],
                                kv_bf[:, h * 128 : (h + 1) * 128],
                                qf[:, h, :],
                                start=True,
                                stop=True,
                            )
                            bc = psum_b.tile([128, BLK], fp32)
                            nc.tensor.matmul(
                                bc[:],
                                sel[:, h * 128 : (h + 1) * 128],
                                rbf[:],
                                start=True,
                                stop=True,
                            )
                            bcs = t2_pool.tile([128, BLK], fp32, tag="bcs")
                            nc.vector.tensor_scalar_mul(bcs[:], bc[:], AS)
                            nc.vector.tensor_mul(an[:, h, :], ap_[:], bcs[:])

                        for c in range(CPB):
                            osb = out_pool.tile([128, 1024], fp32)
                            for half in range(2):
                                opa = psum_o.tile([128, 512], fp32, tag="opa")
                                for h in range(0, H, 2):
                                    nc.tensor.matmul(
                                        opa[:],
                                        an[:, h : h + 2, c * 128 : (c + 1) * 128],
                                        wo[:, h : h + 2, half * 512 : (half + 1) * 512],
                                        start=(h == 0),
                                        stop=(h == 6),
                                        perf_mode=DR,
                                    )
                                opn = psum_o.tile([128, 512], fp32, tag="opn")
                                for d in range(8):
                                    nc.tensor.matmul(
                                        opn[:],
                                        xT[b][:, d, c * 128 : (c + 1) * 128],
                                        w1n[:, d, half * 512 : (half + 1) * 512],
                                        start=(d == 0),
                                        stop=(d == 7),
                                    )
                                oat = t2_pool.tile([128, 512], fp32, tag="oat")
                                nc.vector.tensor_scalar_mul(
                                    oat[:], opa[:], 1.0 / (WS * AS)
                                )
                                nc.vector.tensor_add(
                                    osb[:, half * 512 : (half + 1) * 512],
                                    opn[:],
                                    oat[:],
                                )
                            r0 = b * BLK + c * 128
                            nc.sync.dma_start(out[r0 : r0 + 128, :], osb[:])

        for rep in range(reps):
            emit_rep(rep)

    nc.compile()
    return nc


def _to_bf16(a):
    import ml_dtypes

    return np.ascontiguousarray(np.asarray(a, dtype=np.float32)).astype(
        ml_dtypes.bfloat16
    )


def _to_fp8(a, scale):
    import ml_dtypes

    return np.ascontiguousarray(
        np.asarray(a, dtype=np.float32) * scale
    ).astype(ml_dtypes.float8_e4m3)


def _host_inputs(x, W1, Wv, Wo):
    return {
        "x": _to_bf16(x),
        "w1k8": _to_fp8(np.asarray(W1)[:, DQK : 2 * DQK], WS),
        "w1q8": _to_fp8(np.asarray(W1)[:, 0:DQK], WS),
        "wv8": _to_fp8(Wv, WS),
        "wo8": _to_fp8(Wo, WS),
        "w1na": _to_bf16(np.asarray(W1)[:, 2 * DQK :]),
    }


def kernel(x, W1, b1, Wv, bv, Wo, bo):
    from concourse.bass_utils import run_bass_kernel_spmd

    if "nc" not in _cache:
        _cache["nc"] = _build_bass()
    nc = _cache["nc"]

    hin = _host_inputs(x, W1, Wv, Wo)
    in_maps = []
    for i in range(NCORES):
        m = dict(hin)
        m["x"] = hin["x"][i * NS : (i + 1) * NS]
        in_maps.append(m)
    res = run_bass_kernel_spmd(nc, in_maps, list(range(NCORES)))
    _cache["last_results"] = res
    return np.concatenate([res.results[i]["out"] for i in range(NCORES)], axis=0)


def _make_sharded(nc):
    """Compile the NEFF as a fast-dispatch sharded jit over 8 cores."""
    import jax
    from jax.experimental.shard_map import shard_map
    from jax.sharding import Mesh, PartitionSpec
    from concourse import bass2jax, mybir

    partition_name = nc.partition_id_tensor.name if nc.partition_id_tensor else None
    in_names, out_names, out_avals, zero_outs = [], [], [], []
    for alloc in nc.m.functions[0].allocations:
        if not isinstance(alloc, mybir.MemoryLocationSet):
            continue
        name = alloc.memorylocations[0].name
        if alloc.kind == "ExternalInput":
            if name != partition_name:
                in_names.append(name)
        elif alloc.kind == "ExternalOutput":
            out_names.append(name)
            shape = tuple(alloc.tensor_shape)
            dtype = mybir.dt.np(alloc.dtype)
            out_avals.append(jax.core.ShapedArray(shape, dtype))
            zero_outs.append(np.zeros(shape, dtype))
    all_names = list(in_names) + list(out_names)
    if partition_name is not None:
        all_names.append(partition_name)

    def _body(*args):
        operands = list(args)
        if partition_name is not None:
            operands.append(bass2jax.partition_id_tensor())
        return tuple(
            bass2jax._bass_exec_p.bind(
                *operands,
                out_avals=tuple(out_avals),
                in_names=tuple(all_names),
                out_names=tuple(out_names),
                lowering_input_output_aliases=(),
                sim_require_finite=True,
                sim_require_nnan=True,
                nc=nc,
            )
        )

    devices = jax.devices()[:NCORES]
    mesh = Mesh(np.asarray(devices), ("core",))
    nspec = len(in_names) + len(out_names)
    jitted = jax.jit(
        shard_map(
            _body,
            mesh=mesh,
            in_specs=(PartitionSpec("core"),) * nspec,
            out_specs=(PartitionSpec("core"),) * len(out_names),
            check_rep=False,
        ),
        keep_unused=True,
    )
    return jitted, in_names, zero_outs, mesh


def benchmark(x, W1, b1, Wv, bv, Wo, bo, iters=30, reps=5, trials=5):
    """Measure the steady-state HW execution time of one kernel pass.

    The axon-tunneled dispatch path has a fixed ~2ms per-launch overhead
    (measured: a trivial 2-DMA NEFF costs the same per call as this kernel),
    so wall-clock per call cannot resolve sub-ms kernel times. We therefore
    compile two NEFFs -- the kernel body once (R=1) and unrolled `reps` times
    back-to-back (R=reps) -- and report the marginal cost of one extra pass:
        hw_exec = (t_R - t_1) / (reps - 1)
    computed per trial from queued batches of `iters` calls each
    (fast-dispatch compiled, inputs resident on device); the median over
    `trials` interleaved trials is reported to suppress tunnel jitter.

    Returns (t1_s, tR_s, hw_exec_s).
    """
    import time

    import jax
    from jax.sharding import NamedSharding, PartitionSpec
    from concourse import bass2jax

    bass2jax.install_neuronx_cc_hook()

    hin = _host_inputs(x, W1, Wv, Wo)
    per_in = {}
    for k, v in hin.items():
        per_in[k] = v if k == "x" else np.tile(v, (NCORES, 1))

    def make(nc):
        jitted, in_names, zero_outs, mesh = _make_sharded(nc)
        sh = NamedSharding(mesh, PartitionSpec("core"))
        args = [jax.device_put(per_in[n], sh) for n in in_names]
        args += [
            jax.device_put(
                np.zeros((NCORES * z.shape[0], *z.shape[1:]), z.dtype), sh
            )
            for z in zero_outs
        ]
        compiled = bass2jax.fast_dispatch_compile(
            lambda: jitted.lower(*args).compile()
        )
        for _ in range(3):
            r = compiled(*args)
        jax.block_until_ready(r)
        return compiled, args

    def batch(compiled, args):
        t0 = time.perf_counter()
        rs = [compiled(*args) for _ in range(iters)]
        jax.block_until_ready(rs)
        return (time.perf_counter() - t0) / iters

    if "nc" not in _cache:
        _cache["nc"] = _build_bass()
    b1_ = make(_cache["nc"])
    bR_ = make(_build_bass(reps=reps))
    t1s, tRs, slopes = [], [], []
    for _ in range(trials):
        t1 = batch(*b1_)
        tR = batch(*bR_)
        t1s.append(t1)
        tRs.append(tR)
        slopes.append((tR - t1) / (reps - 1))
    slopes.sort()
    hw = slopes[len(slopes) // 2]
    return min(t1s), min(tRs), hw


# revision 29
# speedup vs baseline: 1.0387x; 1.0387x over previous
"""Trainium2 Bass kernel for KernelSelfAttn (linear attention) distributed over 8 cores.

Math (per reference):
  h1 = x@W1 + b1 ; q,k = h1[:, :1024], h1[:, 1024:2048]; non_att = h1[:, 2048:]
  v = x@Wv + bv
  per head (8 heads, dh=dv=128):
    qf = elu(q)+1 = exp(min(q,0)) + relu(q)   (same for k)
    kv = kf^T @ v ; k_sum = kf.sum(n)         -> reductions over N (all-reduced)
    att = (qf @ kv) / (qf @ k_sum)
  out = non_att + att_cat @ Wo + bo           (biases are all zero per spec)

Sharding: rows of x split across 8 cores; [kv | k_sum] ([128, 1032] fp32)
all-reduced; everything else local.

Precision strategy: the output is non_att + att@Wo where the att contribution
is ~1e-3 of the non_att scale, so the whole attention path tolerates fp8.
 - non_att gemm (x@W1na): bf16 lhsT/rhs (precision-critical).
 - k, v, q gemms and att@Wo: fp8e4 operands with DoubleRow perf mode
   (2 contraction rows/cycle). fp8 weights are host-prescaled by 32 (W ~
   N(0,1/1024) would be subnormal in e4m3); the 1/32 descale folds into the
   activation-engine ops that already run. att@Wo additionally scales the
   normalizer by 1024 so att(~7e-4) lands in e4m3's normal range; the
   combined 1/(32*1024) descale folds into the PSUM-evacuation copy.
 - kv/k_sum/att/qk matmuls stay bf16 (small, and keep the feature pipeline
   simple). All accumulation is fp32 in PSUM; AllReduce in fp32.

Layouts on chip:
  xT   [din-part, n]  bf16 + fp8 copies, SBUF-resident (96KB/part)
  k,v  [n-part, dim]  natural (contraction for kv needs n on partitions)
  qT   [dqk-part, n]  transposed (contraction for att needs dh on partitions)
  attT [dv-part, n]   feeds output projection as stationary operand
"""

import sys

import numpy as np

sys.path.insert(0, "/opt/trn_rl_repo")

DIN = 1024
DQK = 1024
DV = 1024
H = 8
DH = 128
NCORES = 8
N_FULL = 32768
NS = N_FULL // NCORES  # 4096 rows per core
BLK = 512
NBLK = NS // BLK  # 8
CPB = BLK // 128  # chunks (of 128 rows) per block

WS = 32.0  # host pre-scale on fp8 weights
AS = 1024.0  # normalizer pre-scale so att fits e4m3
_cache = {}


def _build_bass(no_collective=False, reps=1):
    """reps>1 unrolls the whole kernel body back-to-back inside one NEFF.
    Used by the benchmark to measure the marginal (steady-state) HW cost of
    one kernel execution, net of the fixed per-launch dispatch overhead."""
    import concourse.bass as bass
    import concourse.mybir as mybir
    import concourse.tile as tile
    from concourse import bacc
    from concourse.masks import make_identity
    from contextlib import ExitStack

    fp32 = mybir.dt.float32
    bf16 = mybir.dt.bfloat16
    fp8 = mybir.dt.float8e4
    AF = mybir.ActivationFunctionType
    ALU = mybir.AluOpType
    DR = mybir.MatmulPerfMode.DoubleRow

    nc = bacc.Bacc(None)

    x = nc.declare_dram_parameter("x", [NS, DIN], bf16, isOutput=False)
    w1k8 = nc.declare_dram_parameter("w1k8", [DIN, DQK], fp8, isOutput=False)
    w1q8 = nc.declare_dram_parameter("w1q8", [DIN, DQK], fp8, isOutput=False)
    wv8 = nc.declare_dram_parameter("wv8", [DIN, DV], fp8, isOutput=False)
    wo8 = nc.declare_dram_parameter("wo8", [DV, DIN], fp8, isOutput=False)
    w1na = nc.declare_dram_parameter("w1na", [DIN, DIN], bf16, isOutput=False)
    out = nc.declare_dram_parameter("out", [NS, DIN], fp32, isOutput=True)

    KVW = H * DH + H  # 1032: [kv (8*128) | k_sum (8)]

    with ExitStack() as top:
        tc = top.enter_context(tile.TileContext(nc))

        consts = top.enter_context(tc.tile_pool(name="consts", bufs=1))
        ident = consts.tile([128, 128], bf16)
        make_identity(nc, ident[:])
        ones = consts.tile([128, 1], bf16)
        nc.gpsimd.memset(ones[:], 1.0)

        # row-selector weights: sel[:, h*128:(h+1)*128] is [8,128] with row h
        # all-ones -> K=8 matmul broadcasts rall[h, :] across 128 partitions
        sel = consts.tile([8, H * 128], bf16)
        sel_i = consts.tile([8, H * 128], mybir.dt.int32)
        nc.gpsimd.iota(
            sel_i[:].rearrange("p (h w) -> p h w", w=128),
            pattern=[[1, H], [0, 128]],
            base=0,
            channel_multiplier=-1,
        )
        nc.vector.tensor_scalar(sel[:], sel_i[:], 0, None, ALU.is_equal)

        dram = top.enter_context(tc.tile_pool(name="dram", bufs=1, space="DRAM"))
        kv_in = dram.tile([128, KVW], fp32)
        kv_out = dram.tile([128, KVW], fp32)

        def emit_rep(rep):
            with ExitStack() as body:
                # xT tiles: one per block, written in phase 1, read by both
                # phases. bf16 copy feeds non_att; fp8 copy feeds k/v/q gemms.
                xt_pool = body.enter_context(tc.tile_pool(name="xt", bufs=1))
                xT = [
                    xt_pool.tile([128, 8, BLK], bf16, name=f"xT{b}", tag=f"xT{b}")
                    for b in range(NBLK)
                ]
                xT8 = [
                    xt_pool.tile([128, 8, BLK], fp8, name=f"xT8_{b}", tag=f"xT8_{b}")
                    for b in range(NBLK)
                ]

                # all weight loads issued up front so phase 2 never waits on DMA
                w_pool = body.enter_context(tc.tile_pool(name="wts", bufs=1))
                w1k = w_pool.tile([128, 8, 1024], fp8, name="w1k", tag="w1k")
                wv = w_pool.tile([128, 8, 1024], fp8, name="wv", tag="wv")
                w1q = w_pool.tile([128, 8, 1024], fp8, name="w1q", tag="w1q")
                w1n = w_pool.tile([128, 8, 1024], bf16, name="w1n", tag="w1n")
                wo = w_pool.tile([128, 8, 1024], fp8, name="wo", tag="wo")
                for d in range(8):
                    nc.sync.dma_start(w1k[:, d, :], w1k8[d * 128 : (d + 1) * 128, :])
                    nc.sync.dma_start(wv[:, d, :], wv8[d * 128 : (d + 1) * 128, :])
                    nc.sync.dma_start(w1q[:, d, :], w1q8[d * 128 : (d + 1) * 128, :])
                    nc.sync.dma_start(w1n[:, d, :], w1na[d * 128 : (d + 1) * 128, :])
                    nc.sync.dma_start(wo[:, d, :], wo8[d * 128 : (d + 1) * 128, :])

                # ---------------- Phase 1: xT, k, v, kv ----------------
                with ExitStack() as p1:

                    # kv / k_sum accumulators live in PSUM across all of phase 1
                    psum_kv = p1.enter_context(
                        tc.tile_pool(name="psum_kv", bufs=1, space="PSUM")
                    )
                    kv_acc = [
                        psum_kv.tile([128, 512], fp32, name="kva0", tag="kva0"),
                        psum_kv.tile([128, 512], fp32, name="kva1", tag="kva1"),
                    ]
                    ks_acc = psum_kv.tile([128, 8], fp32, name="ksa", tag="ksa")

                    xin_pool = p1.enter_context(tc.tile_pool(name="xin", bufs=8))
                    kf_pool = p1.enter_context(tc.tile_pool(name="kfeat", bufs=2))
                    va_pool = p1.enter_context(tc.tile_pool(name="vaug", bufs=2))
                    t1_pool = p1.enter_context(tc.tile_pool(name="p1tmp", bufs=4))
                    psum_t = p1.enter_context(
                        tc.tile_pool(name="psum_t", bufs=2, space="PSUM")
                    )
                    psum_g = p1.enter_context(
                        tc.tile_pool(name="psum_g", bufs=3, space="PSUM")
                    )

                    for b in range(NBLK):
                        xin = []
                        for c in range(CPB):
                            t = xin_pool.tile([128, DIN], bf16)
                            r0 = b * BLK + c * 128
                            nc.sync.dma_start(t[:], x[r0 : r0 + 128, :])
                            xin.append(t)

                        for d in range(8):
                            tp = psum_t.tile([128, BLK], bf16)
                            for c in range(CPB):
                                nc.tensor.transpose(
                                    tp[:, c * 128 : (c + 1) * 128],
                                    xin[c][:, d * 128 : (d + 1) * 128],
                                    ident[:],
                                )
                            nc.vector.tensor_copy(xT[b][:, d, :], tp[:])
                            nc.gpsimd.tensor_copy(xT8[b][:, d, :], xT[b][:, d, :])

                        for c in range(CPB):
                            kf = kf_pool.tile([128, 1024], bf16)
                            va = va_pool.tile([128, 1024], bf16)
                            for s in range(2):  # k halves (psum holds 32*k)
                                ps = psum_g.tile([128, 512], fp32)
                                for d in range(0, 8, 2):
                                    nc.tensor.matmul(
                                        ps[:],
                                        xT8[b][:, d : d + 2, c * 128 : (c + 1) * 128],
                                        w1k[:, d : d + 2, s * 512 : (s + 1) * 512],
                                        start=(d == 0),
                                        stop=(d == 6),
                                        perf_mode=DR,
                                    )
                                # feature map: exp(min(k,0)) + max(k,0), k = ps/32
                                tmp = t1_pool.tile([128, 512], bf16, tag="texp")
                                nc.scalar.activation(
                                    tmp[:], ps[:], AF.Relu, scale=-1.0 / WS
                                )
                                nc.scalar.activation(tmp[:], tmp[:], AF.Exp, scale=-1.0)
                                mx = t1_pool.tile([128, 512], bf16, tag="tmax")
                                nc.scalar.activation(
                                    mx[:], ps[:], AF.Relu, scale=1.0 / WS
                                )
                                nc.vector.tensor_add(
                                    kf[:, s * 512 : (s + 1) * 512], mx[:], tmp[:]
                                )
                            for s in range(2):  # v halves (psum holds 32*v)
                                ps = psum_g.tile([128, 512], fp32)
                                for d in range(0, 8, 2):
                                    nc.tensor.matmul(
                                        ps[:],
                                        xT8[b][:, d : d + 2, c * 128 : (c + 1) * 128],
                                        wv[:, d : d + 2, s * 512 : (s + 1) * 512],
                                        start=(d == 0),
                                        stop=(d == 6),
                                        perf_mode=DR,
                                    )
                                nc.vector.tensor_scalar_mul(
                                    va[:, s * 512 : (s + 1) * 512], ps[:], 1.0 / WS
                                )
                            first = b == 0 and c == 0
                            last = b == NBLK - 1 and c == CPB - 1
                            for h in range(H):
                                nc.tensor.matmul(
                                    kv_acc[h // 4][
                                        :, (h % 4) * 128 : (h % 4 + 1) * 128
                                    ],
                                    kf[:, h * 128 : (h + 1) * 128],
                                    va[:, h * 128 : (h + 1) * 128],
                                    start=first,
                                    stop=last,
                                )
                                nc.tensor.matmul(
                                    ks_acc[:, h : h + 1],
                                    kf[:, h * 128 : (h + 1) * 128],
                                    ones[:],
                                    start=first,
                                    stop=last,
                                )

                    # evacuate [kv | k_sum] to DRAM bounce buffer
                    kv_sb = kf_pool.tile([128, KVW], fp32, name="kv_sb", tag="kv_sb")
                    nc.vector.tensor_copy(kv_sb[:, 0:512], kv_acc[0][:])
                    nc.vector.tensor_copy(kv_sb[:, 512:1024], kv_acc[1][:])
                    nc.vector.tensor_copy(kv_sb[:, 1024:1032], ks_acc[:])
                    nc.sync.dma_start(kv_in[:], kv_sb[:])

                # ---------------- AllReduce [kv | k_sum] ----------------
                if no_collective:  # timeline-sim variant: local copy, no AllReduce
                    nc.sync.dma_start(kv_out[:], kv_in[:])
                else:
                    nc.gpsimd.collective_compute(
                        "AllReduce",
                        mybir.AluOpType.add,
                        replica_groups=[list(range(NCORES))],
                        ins=[kv_in.opt()],
                        outs=[kv_out.opt()],
                    )
                kvp = body.enter_context(tc.tile_pool(name="kvpost", bufs=1))
                kv2 = kvp.tile([128, KVW], fp32, name="kv2", tag="kv2")
                nc.sync.dma_start(kv2[:], kv_out[:])
                kv_bf = kvp.tile([128, 1024], bf16, name="kv_bf", tag="kv_bf")
                nc.vector.tensor_copy(kv_bf[:], kv2[:, 0:1024])

                # block-diagonal k_sum for the qk matmul: ks[:, h*8+h] = k_sum_h
                ks_sb = kvp.tile([128, 64], bf16, name="ks_sb", tag="ks_sb")
                nc.gpsimd.memset(ks_sb[:], 0.0)
                for h in range(H):
                    nc.vector.tensor_copy(
                        ks_sb[:, h * 8 + h : h * 8 + h + 1],
                        kv2[:, 1024 + h : 1024 + h + 1],
                    )

                # ---------------- Phase 2: q, att, out ----------------
                with ExitStack() as p2:
                    qf_pool = p2.enter_context(tc.tile_pool(name="qf", bufs=2))
                    an_pool = p2.enter_context(tc.tile_pool(name="an", bufs=2))
                    t2_pool = p2.enter_context(tc.tile_pool(name="p2tmp", bufs=3))
                    rr_pool = p2.enter_context(tc.tile_pool(name="rall", bufs=2))
                    out_pool = p2.enter_context(tc.tile_pool(name="osb", bufs=2))
                    # psum_q slots shared by q-gemm and the out-gemm's non_att
                    # accumulator; psum_a slots shared by the att matmul and
                    # the out-gemm's att accumulator (disjoint lifetimes).
                    psum_q = p2.enter_context(
                        tc.tile_pool(name="psum_q", bufs=2, space="PSUM")
                    )
                    psum_k = p2.enter_context(
                        tc.tile_pool(name="psum_k", bufs=1, space="PSUM")
                    )
                    psum_a = p2.enter_context(
                        tc.tile_pool(name="psum_a", bufs=2, space="PSUM")
                    )
                    psum_o = p2.enter_context(
                        tc.tile_pool(name="psum_o", bufs=1, space="PSUM")
                    )
                    psum_b = p2.enter_context(
                        tc.tile_pool(name="psum_b", bufs=1, space="PSUM")
                    )

                    for b in range(NBLK):
                        qf = qf_pool.tile([128, H, BLK], bf16)  # [p(dh), head, n]
                        for qh in range(H):
                            qp = psum_q.tile([128, BLK], fp32, tag="qo")
                            for d in range(0, 8, 2):
                                nc.tensor.matmul(
                                    qp[:],
                                    w1q[:, d : d + 2, qh * 128 : (qh + 1) * 128],
                                    xT8[b][:, d : d + 2, :],
                                    start=(d == 0),
                                    stop=(d == 6),
                                    perf_mode=DR,
                                )
                            tmp = t2_pool.tile([128, BLK], bf16, tag="texp")
                            nc.scalar.activation(tmp[:], qp[:], AF.Relu, scale=-1.0 / WS)
                            nc.scalar.activation(tmp[:], tmp[:], AF.Exp, scale=-1.0)
                            mx = t2_pool.tile([128, BLK], bf16, tag="tmax")
                            nc.scalar.activation(mx[:], qp[:], AF.Relu, scale=1.0 / WS)
                            nc.vector.tensor_add(qf[:, qh, :], mx[:], tmp[:])

                        qkp = psum_k.tile([8, BLK], fp32)
                        for h in range(H):
                            nc.tensor.matmul(
                                qkp[:],
                                ks_sb[:, h * 8 : (h + 1) * 8],
                                qf[:, h, :],
                                start=(h == 0),
                                stop=(h == H - 1),
                            )
                        rall = rr_pool.tile([8, BLK], fp32, tag="rall_f")
                        nc.vector.reciprocal(rall[:], qkp[:])
                        rbf = rr_pool.tile([8, BLK], bf16, tag="rall_b")
                        nc.vector.tensor_copy(rbf[:], rall[:])

                        # attT, normalized and pre-scaled by AS, cast to fp8
                        an = an_pool.tile([128, H, BLK], fp8)
                        for h in range(H):
                            ap_ = psum_a.tile([128, BLK], fp32, tag="att")
                            nc.tensor.matmul(
                                ap_[:],
                                kv_bf[:, h * 128 : (h + 1) * 128],
                                qf[:, h, :],
                                start=True,
                                stop=True,
                            )
                            bc = psum_b.tile([128, BLK], fp32)
                            nc.tensor.matmul(
                                bc[:],
                                sel[:, h * 128 : (h + 1) * 128],
                                rbf[:],
                                start=True,
                                stop=True,
                            )
                            bcs = t2_pool.tile([128, BLK], bf16, tag="bcs")
                            nc.vector.tensor_scalar_mul(bcs[:], bc[:], AS)
                            nc.vector.tensor_mul(an[:, h, :], ap_[:], bcs[:])

                        for c in range(CPB):
                            osb = out_pool.tile([128, 1024], fp32)
                            for half in range(2):
                                opa = psum_o.tile([128, 512], fp32, tag="opa")
                                for h in range(0, H, 2):
                                    nc.tensor.matmul(
                                        opa[:],
                                        an[:, h : h + 2, c * 128 : (c + 1) * 128],
                                        wo[:, h : h + 2, half * 512 : (half + 1) * 512],
                                        start=(h == 0),
                                        stop=(h == 6),
                                        perf_mode=DR,
                                    )
                                opn = psum_o.tile([128, 512], fp32, tag="opn")
                                for d in range(8):
                                    nc.tensor.matmul(
                                        opn[:],
                                        xT[b][:, d, c * 128 : (c + 1) * 128],
                                        w1n[:, d, half * 512 : (half + 1) * 512],
                                        start=(d == 0),
                                        stop=(d == 7),
                                    )
                                oat = t2_pool.tile([128, 512], bf16, tag="oat")
                                nc.vector.tensor_scalar_mul(
                                    oat[:], opa[:], 1.0 / (WS * AS)
                                )
                                nc.vector.tensor_add(
                                    osb[:, half * 512 : (half + 1) * 512],
                                    opn[:],
                                    oat[:],
                                )
                            r0 = b * BLK + c * 128
                            nc.sync.dma_start(out[r0 : r0 + 128, :], osb[:])

        for rep in range(reps):
            emit_rep(rep)

    nc.compile()
    return nc


def _to_bf16(a):
    import ml_dtypes

    return np.ascontiguousarray(np.asarray(a, dtype=np.float32)).astype(
        ml_dtypes.bfloat16
    )


def _to_fp8(a, scale):
    import ml_dtypes

    return np.ascontiguousarray(
        np.asarray(a, dtype=np.float32) * scale
    ).astype(ml_dtypes.float8_e4m3)


def _host_inputs(x, W1, Wv, Wo):
    return {
        "x": _to_bf16(x),
        "w1k8": _to_fp8(np.asarray(W1)[:, DQK : 2 * DQK], WS),
        "w1q8": _to_fp8(np.asarray(W1)[:, 0:DQK], WS),
        "wv8": _to_fp8(Wv, WS),
        "wo8": _to_fp8(Wo, WS),
        "w1na": _to_bf16(np.asarray(W1)[:, 2 * DQK :]),
    }


def kernel(x, W1, b1, Wv, bv, Wo, bo):
    from concourse.bass_utils import run_bass_kernel_spmd

    if "nc" not in _cache:
        _cache["nc"] = _build_bass()
    nc = _cache["nc"]

    hin = _host_inputs(x, W1, Wv, Wo)
    in_maps = []
    for i in range(NCORES):
        m = dict(hin)
        m["x"] = hin["x"][i * NS : (i + 1) * NS]
        in_maps.append(m)
    res = run_bass_kernel_spmd(nc, in_maps, list(range(NCORES)))
    _cache["last_results"] = res
    return np.concatenate([res.results[i]["out"] for i in range(NCORES)], axis=0)


def _make_sharded(nc):
    """Compile the NEFF as a fast-dispatch sharded jit over 8 cores."""
    import jax
    from jax.experimental.shard_map import shard_map
    from jax.sharding import Mesh, PartitionSpec
    from concourse import bass2jax, mybir

    partition_name = nc.partition_id_tensor.name if nc.partition_id_tensor else None
    in_names, out_names, out_avals, zero_outs = [], [], [], []
    for alloc in nc.m.functions[0].allocations:
        if not isinstance(alloc, mybir.MemoryLocationSet):
            continue
        name = alloc.memorylocations[0].name
        if alloc.kind == "ExternalInput":
            if name != partition_name:
                in_names.append(name)
        elif alloc.kind == "ExternalOutput":
            out_names.append(name)
            shape = tuple(alloc.tensor_shape)
            dtype = mybir.dt.np(alloc.dtype)
            out_avals.append(jax.core.ShapedArray(shape, dtype))
            zero_outs.append(np.zeros(shape, dtype))
    all_names = list(in_names) + list(out_names)
    if partition_name is not None:
        all_names.append(partition_name)

    def _body(*args):
        operands = list(args)
        if partition_name is not None:
            operands.append(bass2jax.partition_id_tensor())
        return tuple(
            bass2jax._bass_exec_p.bind(
                *operands,
                out_avals=tuple(out_avals),
                in_names=tuple(all_names),
                out_names=tuple(out_names),
                lowering_input_output_aliases=(),
                sim_require_finite=True,
                sim_require_nnan=True,
                nc=nc,
            )
        )

    devices = jax.devices()[:NCORES]
    mesh = Mesh(np.asarray(devices), ("core",))
    nspec = len(in_names) + len(out_names)
    jitted = jax.jit(
        shard_map(
            _body,
            mesh=mesh,
            in_specs=(PartitionSpec("core"),) * nspec,
            out_specs=(PartitionSpec("core"),) * len(out_names),
            check_rep=False,
        ),
        keep_unused=True,
    )
    return jitted, in_names, zero_outs, mesh


def benchmark(x, W1, b1, Wv, bv, Wo, bo, iters=30, reps=5, trials=5):
    """Measure the steady-state HW execution time of one kernel pass.

    The axon-tunneled dispatch path has a fixed ~2ms per-launch overhead
    (measured: a trivial 2-DMA NEFF costs the same per call as this kernel),
    so wall-clock per call cannot resolve sub-ms kernel times. We therefore
    compile two NEFFs -- the kernel body once (R=1) and unrolled `reps` times
    back-to-back (R=reps) -- and report the marginal cost of one extra pass:
        hw_exec = (t_R - t_1) / (reps - 1)
    computed per trial from queued batches of `iters` calls each
    (fast-dispatch compiled, inputs resident on device); the median over
    `trials` interleaved trials is reported to suppress tunnel jitter.

    Returns (t1_s, tR_s, hw_exec_s).
    """
    import time

    import jax
    from jax.sharding import NamedSharding, PartitionSpec
    from concourse import bass2jax

    bass2jax.install_neuronx_cc_hook()

    hin = _host_inputs(x, W1, Wv, Wo)
    per_in = {}
    for k, v in hin.items():
        per_in[k] = v if k == "x" else np.tile(v, (NCORES, 1))

    def make(nc):
        jitted, in_names, zero_outs, mesh = _make_sharded(nc)
        sh = NamedSharding(mesh, PartitionSpec("core"))
        args = [jax.device_put(per_in[n], sh) for n in in_names]
        args += [
            jax.device_put(
                np.zeros((NCORES * z.shape[0], *z.shape[1:]), z.dtype), sh
            )
            for z in zero_outs
        ]
        compiled = bass2jax.fast_dispatch_compile(
            lambda: jitted.lower(*args).compile()
        )
        for _ in range(3):
            r = compiled(*args)
        jax.block_until_ready(r)
        return compiled, args

    def batch(compiled, args):
        t0 = time.perf_counter()
        rs = [compiled(*args) for _ in range(iters)]
        jax.block_until_ready(rs)
        return (time.perf_counter() - t0) / iters

    if "nc" not in _cache:
        _cache["nc"] = _build_bass()
    b1_ = make(_cache["nc"])
    bR_ = make(_build_bass(reps=reps))
    t1s, tRs, slopes = [], [], []
    for _ in range(trials):
        t1 = batch(*b1_)
        tR = batch(*bR_)
        t1s.append(t1)
        tRs.append(tR)
        slopes.append((tR - t1) / (reps - 1))
    slopes.sort()
    hw = slopes[len(slopes) // 2]
    return min(t1s), min(tRs), hw
